# revision 19
# baseline (speedup 1.0000x reference)
"""CSWin-style cross-attention block for Trainium2 (Bass/Tile), 8-core data-parallel.

v3: transposed attention tail to unload the PE (v2 was PE-bound at ~444us of
matmul streaming; probes showed ACT exp ~266us is the real floor):
  - scores S^T per (window, head, kchunk) + ACT exp: unchanged from v2.
  - AV+den: exp blocks [128k, 128q] are the STATIONARY weights; moving data is
    [V^T | ones] (33 cols per head) so one matmul per (qc, h, kc) yields
    att^T[q, hd] AND the softmax denominator in PSUM col 33h+32. Out free size
    is 33 instead of 512 -> AV+den drop from ~524k to ~70k PE cycles.
  - V^T produced by DMA transpose (XBAR) straight into the [vt|ones] layout;
    ones columns are persistent (memset once).
  - normalize: per-partition denominators -> recip on [128,16], broadcast to
    rdfull via stride-0 AP, two strided tensor_tensor muls -> att_n bf16.
  - att_n transposed back to [ch, tok] by DMA transpose, added to LePE on DVE.
  - LePE 3x3 depthwise conv: unchanged diagonal-matmul approach.
  - PSUM: scores 2x2 banks, attT 2x1 banks, aux (qkv/lepe/proj) 2 banks.
"""
import os
import sys

sys.path.insert(0, "/opt/trn_rl_repo")
import numpy as np
import ml_dtypes

import concourse.bacc as bacc
import concourse.mybir as mybir
import concourse.tile as tile
from concourse.bass import AP
from concourse.bass_utils import run_bass_kernel_spmd
from concourse.masks import make_identity

BF = mybir.dt.bfloat16
F32 = mybir.dt.float32
AF = mybir.ActivationFunctionType
ALU = mybir.AluOpType
SCALE = float(32.0 ** -0.5)

# tap order: (0,0) first so the start=True matmul covers the whole region
TAPS = [(0, 0)] + [(dr, dj) for dr in (-1, 0, 1) for dj in (-1, 0, 1) if (dr, dj) != (0, 0)]

# branch -> (combo, qhalf, kvhalf); combo A = 64x8 windows, B = 8x64
BRANCH = {0: ("A", 0, 0), 1: ("B", 1, 1), 2: ("A", 1, 0), 3: ("B", 0, 1)}

# padded flat window layouts for LePE: (rows, cols, row_pitch, region_base, total)
PAD = {"A": (64, 8, 10, 16, 672), "B": (8, 64, 66, 80, 688)}


def build(nc, debug=False, repeat=1, dyn_loop=0, with_cbias=True, probe=frozenset()):
    xT_d = nc.dram_tensor("xT", [256, 4096], BF, kind="ExternalInput").ap()
    qw = nc.dram_tensor("qw", [256, 768], BF, kind="ExternalInput").ap()
    pw = nc.dram_tensor("pw", [512, 256], BF, kind="ExternalInput").ap()
    dg = nc.dram_tensor("dg", [36, 128, 128], BF, kind="ExternalInput").ap()
    pbT = nc.dram_tensor("pbT", [128, 2], F32, kind="ExternalInput").ap()
    cb = nc.dram_tensor("cb", [128, 4], F32, kind="ExternalInput").ap()
    out_d = nc.dram_tensor("out", [256, 4096], F32, kind="ExternalOutput").ap()
    dbg = {}
    if debug:
        for name, shape in [("d_vto", [128, 528]), ("d_attT0", [128, 264]),
                            ("d_attT1", [128, 264]), ("d_attn", [128, 512]),
                            ("d_attnT", [128, 512]), ("d_rdfull", [128, 512]),
                            ("d_exp", [128, 1024])]:
            dbg[name] = nc.dram_tensor(name, shape, F32, kind="ExternalOutput").ap()

    with tile.TileContext(nc) as tc:
        with tc.sbuf_pool(name="persist", bufs=1) as ps_pool:
            # ---- constants / weights ----
            ident = ps_pool.tile([128, 128], BF, name="ident")
            make_identity(nc, ident)
            ones512 = ps_pool.tile([128, 512], BF, name="ones512")
            nc.vector.memset(ones512, 1.0)
            cbdiag = ps_pool.tile([128, 4 * 128], BF, name="cbdiag")

            qw_t = [ps_pool.tile([128, 768], BF, name=f"qw{i}") for i in range(2)]
            for i in range(2):
                nc.sync.dma_start(qw_t[i], qw[128 * i:128 * (i + 1), :])
            pw_t = [ps_pool.tile([128, 256], BF, name=f"pw{i}") for i in range(4)]
            for i in range(4):
                nc.sync.dma_start(pw_t[i], pw[128 * i:128 * (i + 1), :])
            diag_t = ps_pool.tile([128, 36 * 128], BF, name="diag_t")
            nc.sync.dma_start(diag_t.rearrange("p (t c) -> p t c", t=36),
                              dg.rearrange("t p c -> p t c"))
            pbT_t = ps_pool.tile([128, 2], F32, name="pbT_t")
            nc.sync.dma_start(pbT_t, pbT)
            cb_t = ps_pool.tile([128, 4], F32, name="cb_t")
            nc.sync.dma_start(cb_t, cb)
            for _b in range(4):
                nc.vector.tensor_scalar(cbdiag[:, 128 * _b:128 * (_b + 1)], ident,
                                        cb_t[:, _b:_b + 1], None, ALU.mult)

            # ---- persistent activations ----
            # window-major q/k/v: A layout col = 512w + 8r + j (w=A-window),
            # B layout = row-major tokens (window w = cols 512w..512w+512).
            xT = [ps_pool.tile([128, 4096], BF, name=f"xT{i}") for i in range(2)]
            qA = [ps_pool.tile([128, 4096], BF, name=f"qA{i}") for i in range(2)]
            qB = [ps_pool.tile([128, 4096], BF, name=f"qB{i}") for i in range(2)]
            kA0 = ps_pool.tile([128, 4096], BF, name="kA0")
            vA0 = ps_pool.tile([128, 4096], BF, name="vA0")
            kB1 = ps_pool.tile([128, 4096], BF, name="kB1")
            vB1 = ps_pool.tile([128, 4096], BF, name="vB1")
            cat_t = [ps_pool.tile([128, 4096], BF, name=f"cat{i}") for i in range(4)]
            # persistent zero-padded LePE buffers (pad cells stay zero; the
            # data region is overwritten per window). 2 per combo: ping-pong.
            vpadA = [ps_pool.tile([128, PAD["A"][4]], BF, name=f"vpadA{i}") for i in range(2)]
            vpadB = [ps_pool.tile([128, PAD["B"][4]], BF, name=f"vpadB{i}") for i in range(2)]
            for t in vpadA + vpadB:
                nc.vector.memset(t, 0.0)
            # [V^T | ones] tiles for the transposed AV+den matmuls:
            # [128 tok, 4kc x (4h x 33)]; col 132kc + 33h + i = channel 32h+i
            # of kc block (i<32), col 132kc + 33h + 32 = 1.0 (denominator).
            vtones = [ps_pool.tile([128, 528], BF, name=f"vtones{i}") for i in range(2)]
            for t in vtones:
                nc.vector.memset(t, 0.0)
                nc.vector.memset(
                    t.rearrange("p (k h x) -> p k h x", k=4, h=4)[:, :, :, 32:33], 1.0)
            # ACT exp-table preload at t=0
            warm = ps_pool.tile([128, 1], F32, name="warm")
            nc.scalar.activation(warm, ones512[:, 0:1], AF.Exp, scale=1.0)

            # qkv dest map: m-block -> list of (dest tile, layout)
            #   m: 0=q0 1=q1 2=k0 3=k1 4=v0 5=v1
            qkv_dest = {
                0: [(qB[0], "B"), (qA[0], "A")],
                1: [(qB[1], "B"), (qA[1], "A")],
                2: [(kA0, "A")],
                3: [(kB1, "B")],
                4: [(vA0, "A")],
                5: [(vB1, "B")],
            }

            def _emit(_rep):
                with tc.tile_pool(name=f"scps{_rep}", bufs=2, space="PSUM") as scps, \
                     tc.tile_pool(name=f"atps{_rep}", bufs=1, space="PSUM") as atps, \
                     tc.tile_pool(name=f"auxps{_rep}", bufs=2, space="PSUM") as auxps, \
                     tc.sbuf_pool(name=f"expsb{_rep}", bufs=16) as expsb, \
                     tc.sbuf_pool(name=f"stg{_rep}", bufs=2) as stg, \
                     tc.sbuf_pool(name=f"outsb{_rep}", bufs=4) as outsb:

                    def qkv_chunk(n):
                        """token chunk n (512 tokens): DMA xT cols, 6 QKV matmuls,
                        scatter copies into layout tiles."""
                        for cc in range(2):
                            nc.sync.dma_start(xT[cc][:, 512 * n:512 * (n + 1)],
                                              xT_d[128 * cc:128 * (cc + 1),
                                                   512 * n:512 * (n + 1)])
                        for m in range(6):
                            qp = auxps.tile([128, 512], F32, tag="aux", name="qp")
                            for cc in range(2):
                                nc.tensor.matmul(qp, qw_t[cc][:, 128 * m:128 * (m + 1)],
                                                 xT[cc][:, 512 * n:512 * (n + 1)],
                                                 start=(cc == 0), stop=(cc == 1),
                                                 skip_group_check=True)
                            for dst, layout in qkv_dest[m]:
                                if layout == "B":
                                    nc.vector.tensor_copy(dst[:, 512 * n:512 * (n + 1)], qp)
                                else:
                                    # A scatter: dst col = 512w + 8r + j with
                                    # r = 8n + r8; src col = 64r8 + 8w + j
                                    dv = dst.rearrange("c (w r j) -> c w r j",
                                                       w=8, r=64, j=8)[:, :, 8 * n:8 * (n + 1), :]
                                    sv = qp.rearrange("c (r w j) -> c w r j",
                                                      r=8, w=8, j=8)
                                    nc.vector.tensor_copy(dv, sv)

                    def window_pair(combo, w):
                        R, J, T, RB, TOT = PAD[combo]
                        if combo == "A":
                            branches, kwin_t, vwin_t, vpad_t = (0, 2), kA0, vA0, vpadA[w % 2]
                            qsrc = qA
                        else:
                            branches, kwin_t, vwin_t, vpad_t = (1, 3), kB1, vB1, vpadB[w % 2]
                            qsrc = qB
                        kwin = kwin_t[:, 512 * w:512 * (w + 1)]
                        vwin = vwin_t[:, 512 * w:512 * (w + 1)]
                        vto = vtones[w % 2]

                        # ---- zero-padded v window for LePE (pad stays 0) ----
                        nc.vector.tensor_copy(
                            vpad_t[:, RB:RB + R * T].rearrange(
                                "c (r t) -> c r t", t=T)[:, :, 1:1 + J],
                            vwin.rearrange("c (r j) -> c r j", j=J))

                        # ---- [V^T | ones]: XBAR writes full 128-col tiles, so
                        # transpose each kc block to contiguous scratch, then
                        # one strided DVE copy into the 33-col slots (the ones
                        # columns are persistent).
                        vtsc = stg.tile([128, 512], BF, tag="vtsc", name="vtsc")
                        for kc in range(4):
                            nc.sync.dma_start_transpose(
                                vtsc[:, 128 * kc:128 * (kc + 1)],
                                vwin[:, 128 * kc:128 * (kc + 1)])
                        nc.vector.tensor_copy(
                            vto.rearrange("p (k h x) -> p k h x", k=4, h=4)[:, :, :, 0:32],
                            vtsc.rearrange("p (k h x) -> p k h x", k=4, h=4))

                        # per-branch state for the two-phase pipeline
                        hr = R // 2
                        hspan = hr * T
                        st = {br: {"attT": [None, None], "lps": [None, None],
                                   "exp": {}} for br in branches}

                        def get_attT(br, t):
                            s = st[br]
                            if s["attT"][t] is None:
                                s["attT"][t] = atps.tile([128, 264], F32,
                                                         tag=f"attT{t}", name="attT")
                            return s["attT"][t]

                        def lepe_half(br, half):
                            lp = auxps.tile([128, hspan], F32, tag="aux", name="lp")
                            st[br]["lps"][half] = lp
                            base = RB + half * hspan
                            for t, (dr, dj) in enumerate(TAPS):
                                delta = T * dr + dj
                                dmat = diag_t[:, (br * 9 + t) * 128:(br * 9 + t + 1) * 128]
                                nc.tensor.matmul(
                                    lp,
                                    dmat,
                                    vpad_t[:, base + delta:base + delta + hspan],
                                    start=(t == 0),
                                    stop=(not with_cbias and t == 8),
                                    skip_group_check=True)
                            if with_cbias:
                                nc.tensor.matmul(
                                    lp,
                                    cbdiag[:, 128 * br:128 * (br + 1)],
                                    ones512[:, 0:hspan],
                                    start=False, stop=True, skip_group_check=True)

                        def phase1(br, do_lepe):
                            """scores + exp (kc loop), optionally LePE halves
                            interleaved to keep PE fed while ACT runs exps."""
                            qfull = qsrc[BRANCH[br][1]][:, 512 * w:512 * (w + 1)]
                            exp_tiles = st[br]["exp"]
                            for kc in range(4):
                                sct = [scps.tile([128, 1024], F32, tag="sc", name="sct")
                                       for _ in range(2)]
                                for h in range(4):
                                    nc.tensor.matmul(
                                        sct[h // 2][:, 512 * (h % 2):512 * (h % 2) + 512],
                                        kwin[32 * h:32 * (h + 1), 128 * kc:128 * (kc + 1)],
                                        qfull[32 * h:32 * (h + 1), :],
                                        start=True, stop=True,
                                        tile_position=(32 * h, 0))
                                for p in range(2):
                                    e = expsb.tile([128, 1024], BF, tag="exp", name="exp")
                                    if "act_lite" in probe:
                                        nc.scalar.activation(e[:, :128], sct[p][:, :128],
                                                             AF.Exp, scale=SCALE)
                                    else:
                                        nc.scalar.activation(e, sct[p], AF.Exp, scale=SCALE)
                                    exp_tiles[(p, kc)] = e
                                if do_lepe and kc < 2:
                                    lepe_half(br, kc)

                        def phase2(br):
                            """AV+den (kc-consecutive groups), normalize,
                            transpose back, cat add."""
                            exp_tiles = st[br]["exp"]
                            if "av_off" not in probe:
                                for qc in range(4):
                                    t, u = qc // 2, qc % 2
                                    for h in range(4):
                                        for kc in range(4):
                                            nc.tensor.matmul(
                                                get_attT(br, t)[:, 132 * u + 33 * h:132 * u + 33 * h + 33],
                                                exp_tiles[(h // 2, kc)][:, 512 * (h % 2) + 128 * qc:
                                                                        512 * (h % 2) + 128 * qc + 128],
                                                vto[:, 132 * kc + 33 * h:132 * kc + 33 * h + 33],
                                                start=(kc == 0), stop=(kc == 3),
                                                skip_group_check=True)

                            # --- normalize: att_n[q, 128qc+32h+i] bf16 ---
                            rd16 = stg.tile([128, 16], F32, tag="rd16", name="rd16")
                            for t in range(2):
                                nc.vector.reciprocal_approx_fast(
                                    rd16.rearrange("p (t u h) -> p t u h", t=2, u=2)[:, t],
                                    get_attT(br, t).rearrange("p (u h x) -> p u h x",
                                                              u=2, x=33)[:, :, :, 32])
                            rdfull = stg.tile([128, 512], F32, tag="rdfull", name="rdfull")
                            rd_bc = AP(rd16.tensor, rd16.offset,
                                       [rd16.ap[0], [1, 16], [0, 32]])
                            nc.vector.tensor_copy(
                                rdfull.rearrange("p (q x) -> p q x", q=16), rd_bc)
                            att_n = stg.tile([128, 512], BF, tag="att_n", name="att_n")
                            for t in range(2):
                                nc.vector.tensor_tensor(
                                    att_n.rearrange("p (t u h x) -> p t u h x",
                                                    t=2, u=2, x=32)[:, t],
                                    st[br]["attT"][t].rearrange("p (u h x) -> p u h x",
                                                                u=2, x=33)[:, :, :, 0:32],
                                    rdfull.rearrange("p (t u h x) -> p t u h x",
                                                     t=2, u=2, x=32)[:, t],
                                    ALU.mult)

                            # --- transpose back to [ch, tok] via DMA XBAR ---
                            attn_T = stg.tile([128, 512], BF, tag="attn_T", name="attn_T")
                            for qc in range(4):
                                nc.sync.dma_start_transpose(
                                    attn_T[:, 128 * qc:128 * (qc + 1)],
                                    att_n[:, 128 * qc:128 * (qc + 1)])

                            if debug and combo == "B" and w == 0 and br == 1:
                                def _dump(name, src):
                                    d = stg.tile([128, dbg[name].shape[1]], F32,
                                                 tag="dbg" + name, name="d" + name)
                                    nc.vector.tensor_copy(d, src)
                                    nc.sync.dma_start(dbg[name], d)
                                _dump("d_vto", vto)
                                _dump("d_attT0", st[br]["attT"][0])
                                _dump("d_attT1", st[br]["attT"][1])
                                _dump("d_attn", att_n)
                                _dump("d_attnT", attn_T)
                                _dump("d_rdfull", rdfull)
                                _dump("d_exp", exp_tiles[(0, 0)])

                            # --- cat = attn_T + lepe (unpadded view) ---
                            if combo == "A":
                                catw = cat_t[br].rearrange(
                                    "c (r w j) -> c w r j", r=64, w=8, j=8)[:, w]
                            else:
                                catw = cat_t[br][:, 512 * w:512 * (w + 1)].rearrange(
                                    "c (r j) -> c r j", j=J)
                            for half in range(2):
                                lpv = st[br]["lps"][half].rearrange(
                                    "c (r t) -> c r t", t=T)[:, :, 1:1 + J]
                                t3 = attn_T[:, hr * J * half:hr * J * (half + 1)].rearrange(
                                    "c (a b) -> c a b", a=hr, b=J)
                                nc.vector.tensor_add(catw[:, hr * half:hr * (half + 1), :],
                                                     lpv, t3)

                        brA, brB = branches
                        phase1(brA, do_lepe=True)
                        phase1(brB, do_lepe=False)
                        phase2(brA)
                        lepe_half(brB, 0)
                        lepe_half(brB, 1)
                        phase2(brB)

                    # ============ emission: QKV pipelined with B, then A ============
                    qkv_chunk(0)
                    qkv_chunk(1)
                    for w in range(8):
                        window_pair("B", w)
                        if w + 2 < 8:
                            qkv_chunk(w + 2)
                    for w in range(8):
                        window_pair("A", w)

                    # ============ proj^T tail ============
                    for u, (n, oh) in enumerate([(n, oh) for n in range(8) for oh in range(2)]):
                        pp = auxps.tile([128, 512], F32, tag="aux", name="pp")
                        for b2 in range(4):
                            nc.tensor.matmul(pp, pw_t[b2][:, 128 * oh:128 * (oh + 1)],
                                             cat_t[b2][:, 512 * n:512 * (n + 1)],
                                             start=(b2 == 0), stop=(b2 == 3),
                                             skip_group_check=True)
                        osb = outsb.tile([128, 512], F32, tag="out", name="osb")
                        nc.vector.tensor_scalar(osb, pp, pbT_t[:, oh:oh + 1], None, ALU.add)
                        nc.sync.dma_start(out_d[128 * oh:128 * (oh + 1),
                                                512 * n:512 * (n + 1)], osb)

            if dyn_loop:
                with tc.For_i(0, dyn_loop, 1):
                    _emit(0)
            else:
                for _rep in range(repeat):
                    _emit(_rep)

    return nc


_CACHE = {}


def _get_nc(debug=False, repeat=1, dyn_loop=0, with_cbias=True, probe=frozenset()):
    key = (bool(debug), repeat, dyn_loop, with_cbias, probe)
    if key not in _CACHE:
        nc = bacc.Bacc("TRN2", target_bir_lowering=False, debug=False)
        build(nc, debug=debug, repeat=repeat, dyn_loop=dyn_loop, with_cbias=with_cbias,
              probe=probe)
        nc.compile()
        _CACHE[key] = nc
    return _CACHE[key]


def prep_inputs(x, qkv_w, proj_w, proj_b, conv_ws, conv_bs):
    x = np.asarray(x)
    B = x.shape[0]
    qwb = np.asarray(qkv_w).astype(ml_dtypes.bfloat16)
    pwb = np.asarray(proj_w).astype(ml_dtypes.bfloat16)
    w9 = np.asarray(conv_ws).reshape(4, 128, 9).astype(np.float32)
    dgn = np.zeros((36, 128, 128), np.float32)
    idx = np.arange(128)
    for br in range(4):
        for t, (dr, dj) in enumerate(TAPS):
            dgn[br * 9 + t, idx, idx] = w9[br, :, (dr + 1) * 3 + (dj + 1)]
    dgn = dgn.astype(ml_dtypes.bfloat16)
    pbTn = np.ascontiguousarray(
        np.asarray(proj_b, np.float32).reshape(2, 128).T)
    cbt = np.ascontiguousarray(np.asarray(conv_bs, np.float32).T)
    shared = {"qw": qwb, "pw": pwb, "dg": dgn, "pbT": pbTn, "cb": cbt}
    return [dict(shared,
                 xT=np.ascontiguousarray(x[b].T.astype(ml_dtypes.bfloat16)))
            for b in range(B)]


def kernel(x, qkv_w, proj_w, proj_b, conv_ws, conv_bs, _debug=False, _trace=False):
    wcb = bool(np.any(np.asarray(conv_bs)))
    nc = _get_nc(debug=_debug, with_cbias=wcb)
    in_maps = prep_inputs(x, qkv_w, proj_w, proj_b, conv_ws, conv_bs)
    res = run_bass_kernel_spmd(nc, in_maps, core_ids=list(range(len(in_maps))),
                               trace=_trace)
    out = np.stack([np.ascontiguousarray(r["out"].T) for r in res.results]
                   ).astype(np.float32)
    if _debug or _trace:
        kernel.last_results = res
    return out


# revision 20
# speedup vs baseline: 1.3784x; 1.3784x over previous
"""CSWin-style cross-attention block for Trainium2 (Bass/Tile), 8-core data-parallel.

v2 redesign around the ACT-engine exp bottleneck (~255us of exp is the hard
floor; everything else is arranged to keep ACT gap-free):
  - host passes x pre-transposed [256, 4096]; QKV writes q/k/v directly into
    the window-major layout each consumer needs (A = 64x8 column-strip
    windows for branches 0/2 on kv-half 0; B = 8x64 row-strip windows for
    branches 1/3 on kv-half 1; q needs both layouts).
  - QKV chunks pipelined with combo-B windows (B window w only needs token
    chunk w), then combo A, then a transposed projection tail (out^T
    [256, 4096], host transposes back).
  - scores: S^T per (window, head, kchunk), 4 heads row-packed at array rows
    32h (concurrent on HW); exp on ACT (scale folded), bf16 out.
  - AV: att packed [128 = 4h x 32, 512] via 4-way col tiling; denominator
    separately accumulated with all-ones lhsT, also 4-way col-tiled, so the
    normalize is 3 full-partition DVE ops (recip / mul / add) per branch.
  - LePE 3x3 depthwise conv: 9 diagonal-weight matmuls over persistent
    zero-padded flat window buffers (pad stays zero across windows); output
    split into two half-window tiles of one PSUM bank each so the aux pool
    (lp / qkv psum / proj psum) double-buffers instead of ping-pong-stalling
    the in-order PE queue against DVE copies.
  - proj^T: out^T[oc, tok] = sum_b pw_b[:, oc]^T @ cat_b[:, tok], N=512
    matmuls, bias via DVE tensor_scalar (per-partition).
"""
import os
import sys

sys.path.insert(0, "/opt/trn_rl_repo")
import numpy as np
import ml_dtypes

import concourse.bacc as bacc
import concourse.mybir as mybir
import concourse.tile as tile
from concourse.bass_utils import run_bass_kernel_spmd
from concourse.masks import make_identity

BF = mybir.dt.bfloat16
F32 = mybir.dt.float32
AF = mybir.ActivationFunctionType
ALU = mybir.AluOpType
SCALE = float(32.0 ** -0.5)

# tap order: (0,0) first so the start=True matmul covers the whole region
TAPS = [(0, 0)] + [(dr, dj) for dr in (-1, 0, 1) for dj in (-1, 0, 1) if (dr, dj) != (0, 0)]

# branch -> (combo, qhalf, kvhalf); combo A = 64x8 windows, B = 8x64
BRANCH = {0: ("A", 0, 0), 1: ("B", 1, 1), 2: ("A", 1, 0), 3: ("B", 0, 1)}

# padded flat window layouts for LePE: (rows, cols, row_pitch, region_base, total)
PAD = {"A": (64, 8, 10, 16, 672), "B": (8, 64, 66, 80, 688)}


def build(nc, debug=False, repeat=1, dyn_loop=0, with_cbias=True, probe=frozenset()):
    xT_d = nc.dram_tensor("xT", [256, 4096], BF, kind="ExternalInput").ap()
    qw = nc.dram_tensor("qw", [256, 768], BF, kind="ExternalInput").ap()
    pw = nc.dram_tensor("pw", [512, 256], BF, kind="ExternalInput").ap()
    dg = nc.dram_tensor("dg", [36, 128, 128], BF, kind="ExternalInput").ap()
    pbT = nc.dram_tensor("pbT", [128, 2], F32, kind="ExternalInput").ap()
    cb = nc.dram_tensor("cb", [128, 4], F32, kind="ExternalInput").ap()
    out_d = nc.dram_tensor("out", [256, 4096], F32, kind="ExternalOutput").ap()
    dbg = {}
    if debug:
        for name, shape in [("d_sc", [128, 2048]), ("d_exp", [128, 2048]),
                            ("d_av", [128, 512]), ("d_den", [128, 512]),
                            ("d_lep", [128, 688]), ("d_cat", [128, 512]),
                            ("d_vt", [128, 512]), ("d_q", [128, 512]),
                            ("d_k", [128, 512]), ("d_v", [128, 512])]:
            dbg[name] = nc.dram_tensor(name, shape, F32, kind="ExternalOutput").ap()

    with tile.TileContext(nc) as tc:
        with tc.sbuf_pool(name="persist", bufs=1) as ps_pool:
            # ---- constants / weights ----
            ident = ps_pool.tile([128, 128], BF, name="ident")
            make_identity(nc, ident)
            ones512 = ps_pool.tile([128, 512], BF, name="ones512")
            nc.vector.memset(ones512, 1.0)
            cbdiag = ps_pool.tile([128, 4 * 128], BF, name="cbdiag")

            qw_t = [ps_pool.tile([128, 768], BF, name=f"qw{i}") for i in range(2)]
            for i in range(2):
                nc.sync.dma_start(qw_t[i], qw[128 * i:128 * (i + 1), :])
            pw_t = [ps_pool.tile([128, 256], BF, name=f"pw{i}") for i in range(4)]
            for i in range(4):
                nc.sync.dma_start(pw_t[i], pw[128 * i:128 * (i + 1), :])
            diag_t = ps_pool.tile([128, 36 * 128], BF, name="diag_t")
            nc.sync.dma_start(diag_t.rearrange("p (t c) -> p t c", t=36),
                              dg.rearrange("t p c -> p t c"))
            pbT_t = ps_pool.tile([128, 2], F32, name="pbT_t")
            nc.sync.dma_start(pbT_t, pbT)
            cb_t = ps_pool.tile([128, 4], F32, name="cb_t")
            nc.sync.dma_start(cb_t, cb)
            for _b in range(4):
                nc.vector.tensor_scalar(cbdiag[:, 128 * _b:128 * (_b + 1)], ident,
                                        cb_t[:, _b:_b + 1], None, ALU.mult)

            # ---- persistent activations ----
            # window-major q/k/v: A layout col = 512w + 8r + j (w=A-window),
            # B layout = row-major tokens (window w = cols 512w..512w+512).
            xT = [ps_pool.tile([128, 4096], BF, name=f"xT{i}") for i in range(2)]
            qA = [ps_pool.tile([128, 4096], BF, name=f"qA{i}") for i in range(2)]
            qB = [ps_pool.tile([128, 4096], BF, name=f"qB{i}") for i in range(2)]
            kA0 = ps_pool.tile([128, 4096], BF, name="kA0")
            vA0 = ps_pool.tile([128, 4096], BF, name="vA0")
            kB1 = ps_pool.tile([128, 4096], BF, name="kB1")
            vB1 = ps_pool.tile([128, 4096], BF, name="vB1")
            cat_t = [ps_pool.tile([128, 4096], BF, name=f"cat{i}") for i in range(4)]
            # persistent zero-padded LePE buffers (pad cells stay zero; the
            # data region is overwritten per window). 2 per combo: ping-pong.
            vpadA = [ps_pool.tile([128, PAD["A"][4]], BF, name=f"vpadA{i}") for i in range(2)]
            vpadB = [ps_pool.tile([128, PAD["B"][4]], BF, name=f"vpadB{i}") for i in range(2)]
            for t in vpadA + vpadB:
                nc.vector.memset(t, 0.0)
            # ACT exp-table preload at t=0
            warm = ps_pool.tile([128, 1], F32, name="warm")
            nc.scalar.activation(warm, ones512[:, 0:1], AF.Exp, scale=1.0)

            # qkv dest map: m-block -> list of (dest tile, layout)
            #   m: 0=q0 1=q1 2=k0 3=k1 4=v0 5=v1
            qkv_dest = {
                0: [(qB[0], "B"), (qA[0], "A")],
                1: [(qB[1], "B"), (qA[1], "A")],
                2: [(kA0, "A")],
                3: [(kB1, "B")],
                4: [(vA0, "A")],
                5: [(vB1, "B")],
            }

            def _emit(_rep):
                with tc.tile_pool(name=f"scps{_rep}", bufs=2, space="PSUM") as scps, \
                     tc.tile_pool(name=f"avps{_rep}", bufs=1, space="PSUM") as avps, \
                     tc.tile_pool(name=f"dnps{_rep}", bufs=1, space="PSUM") as dnps, \
                     tc.tile_pool(name=f"auxps{_rep}", bufs=2, space="PSUM") as auxps, \
                     tc.sbuf_pool(name=f"expsb{_rep}", bufs=16) as expsb, \
                     tc.sbuf_pool(name=f"stg{_rep}", bufs=2) as stg, \
                     tc.sbuf_pool(name=f"outsb{_rep}", bufs=4) as outsb:

                    def qkv_chunk(n):
                        """token chunk n (512 tokens): DMA xT cols, 6 QKV matmuls,
                        scatter copies into layout tiles."""
                        for cc in range(2):
                            nc.sync.dma_start(xT[cc][:, 512 * n:512 * (n + 1)],
                                              xT_d[128 * cc:128 * (cc + 1),
                                                   512 * n:512 * (n + 1)])
                        for m in range(6):
                            qp = auxps.tile([128, 512], F32, tag="aux", name="qp")
                            for cc in range(2):
                                nc.tensor.matmul(qp, qw_t[cc][:, 128 * m:128 * (m + 1)],
                                                 xT[cc][:, 512 * n:512 * (n + 1)],
                                                 start=(cc == 0), stop=(cc == 1),
                                                 skip_group_check=True)
                            for dst, layout in qkv_dest[m]:
                                if layout == "B":
                                    nc.vector.tensor_copy(dst[:, 512 * n:512 * (n + 1)], qp)
                                else:
                                    # A scatter: dst col = 512w + 8r + j with
                                    # r = 8n + r8; src col = 64r8 + 8w + j
                                    dv = dst.rearrange("c (w r j) -> c w r j",
                                                       w=8, r=64, j=8)[:, :, 8 * n:8 * (n + 1), :]
                                    sv = qp.rearrange("c (r w j) -> c w r j",
                                                      r=8, w=8, j=8)
                                    nc.vector.tensor_copy(dv, sv)

                    def window_pair(combo, w):
                        R, J, T, RB, TOT = PAD[combo]
                        if combo == "A":
                            branches, kwin_t, vwin_t, vpad_t = (0, 2), kA0, vA0, vpadA[w % 2]
                            qsrc = qA
                        else:
                            branches, kwin_t, vwin_t, vpad_t = (1, 3), kB1, vB1, vpadB[w % 2]
                            qsrc = qB
                        kwin = kwin_t[:, 512 * w:512 * (w + 1)]
                        vwin = vwin_t[:, 512 * w:512 * (w + 1)]
                        is_dbg_w = debug and combo == "A" and w == 0

                        # ---- zero-padded v window for LePE (pad stays 0) ----
                        nc.vector.tensor_copy(
                            vpad_t[:, RB:RB + R * T].rearrange(
                                "c (r t) -> c r t", t=T)[:, :, 1:1 + J],
                            vwin.rearrange("c (r j) -> c r j", j=J))

                        # ---- V^T via PE transposes (shared by branch pair) ----
                        vtp = avps.tile([128, 512], BF, tag="av", name="vtp")
                        for kc in range(4):
                            nc.tensor.transpose(vtp[:, 128 * kc:128 * (kc + 1)],
                                                vwin[:, 128 * kc:128 * (kc + 1)], ident)
                        vt_sb = stg.tile([128, 512], BF, tag="vt", name="vt_sb")
                        nc.vector.tensor_copy(vt_sb, vtp)
                        if is_dbg_w:
                            vt32 = stg.tile([128, 512], F32, tag="dbgvt", name="vt32")
                            nc.vector.tensor_copy(vt32, vt_sb)
                            nc.sync.dma_start(dbg["d_vt"], vt32)

                        # two-phase pipelined emission: scoresB feed ACT
                        # while AV+den of branch A run on PE.
                        st = {br: {"exp": {}, "lps": [None, None]} for br in branches}

                        def lepe_half(br, half):
                            R_, J_, T_, RB_, _ = PAD[combo]
                            hr = R_ // 2
                            hspan = hr * T_
                            lp = auxps.tile([128, hspan], F32, tag="aux", name="lp")
                            st[br]["lps"][half] = lp
                            base = RB_ + half * hspan
                            for t, (dr, dj) in enumerate(TAPS):
                                delta = T_ * dr + dj
                                dmat = diag_t[:, (br * 9 + t) * 128:(br * 9 + t + 1) * 128]
                                nc.tensor.matmul(
                                    lp,
                                    dmat,
                                    vpad_t[:, base + delta:base + delta + hspan],
                                    start=(t == 0),
                                    stop=(not with_cbias and t == 8),
                                    skip_group_check=True)
                            if with_cbias:
                                nc.tensor.matmul(
                                    lp,
                                    cbdiag[:, 128 * br:128 * (br + 1)],
                                    ones512[:, 0:hspan],
                                    start=False, stop=True, skip_group_check=True)

                        def phase1(br, do_lepe):
                            qfull = qsrc[BRANCH[br][1]][:, 512 * w:512 * (w + 1)]
                            exp_tiles = st[br]["exp"]
                            for kc in range(4):
                                sct = [scps.tile([128, 1024], F32, tag="sc", name="sct")
                                       for _ in range(2)]
                                for h in range(4):
                                    nc.tensor.matmul(
                                        sct[h // 2][:, 512 * (h % 2):512 * (h % 2) + 512],
                                        kwin[32 * h:32 * (h + 1), 128 * kc:128 * (kc + 1)],
                                        qfull[32 * h:32 * (h + 1), :],
                                        start=True, stop=True,
                                        tile_position=(32 * h, 0))
                                for p in range(2):
                                    e = expsb.tile([128, 1024], BF, tag="exp", name="exp")
                                    if "act_lite" in probe:
                                        nc.scalar.activation(e[:, :128], sct[p][:, :128],
                                                             AF.Exp, scale=SCALE)
                                    else:
                                        nc.scalar.activation(e, sct[p], AF.Exp, scale=SCALE)
                                    exp_tiles[(p, kc)] = e
                                if do_lepe and kc < 2:
                                    lepe_half(br, kc)

                        def phase2(br):
                            exp_tiles = st[br]["exp"]
                            lps = st[br]["lps"]
                            R_, J_, T_, RB_, _ = PAD[combo]
                            hr = R_ // 2
                            hspan = hr * T_

                            # --- AV (att packed [4h x 32, 512]) + den ---
                            att = avps.tile([128, 512], F32, tag="av", name="att")
                            den = dnps.tile([128, 512], F32, tag="dn", name="den")
                            for kc in range(4):
                                for h in range(4):
                                    nc.tensor.matmul(
                                        att[32 * h:32 * (h + 1), :],
                                        vt_sb[:, 128 * kc + 32 * h:128 * kc + 32 * (h + 1)],
                                        exp_tiles[(h // 2, kc)][:, 512 * (h % 2):512 * (h % 2) + 512],
                                        start=(kc == 0), stop=(kc == 3),
                                        tile_position=(0, 32 * h), skip_group_check=True)
                                for h in range(4):
                                    if "den_lite" in probe and kc > 0:
                                        continue
                                    nc.tensor.matmul(
                                        den[32 * h:32 * (h + 1), :],
                                        ones512[:, 0:32],
                                        exp_tiles[(h // 2, kc)][:, 512 * (h % 2):512 * (h % 2) + 512],
                                        start=(kc == 0),
                                        stop=(kc == 3 or "den_lite" in probe),
                                        tile_position=(0, 32 * h), skip_group_check=True)

                            # --- normalize + lepe -> cat (full-partition ops) ---
                            rd = stg.tile([128, 512], F32, tag="recip", name="rd")
                            nc.vector.reciprocal_approx_fast(rd, den)
                            t_sb = stg.tile([128, 512], F32, tag="tsb", name="t_sb")
                            nc.vector.tensor_mul(t_sb, att, rd)
                            if combo == "A":
                                catw = cat_t[br].rearrange(
                                    "c (r w j) -> c w r j", r=64, w=8, j=8)[:, w]
                            else:
                                catw = cat_t[br][:, 512 * w:512 * (w + 1)].rearrange(
                                    "c (r j) -> c r j", j=J_)
                            for half in range(2):
                                lpv = lps[half].rearrange(
                                    "c (r t) -> c r t", t=T_)[:, :, 1:1 + J_]
                                t3 = t_sb[:, hr * J_ * half:hr * J_ * (half + 1)].rearrange(
                                    "c (a b) -> c a b", a=hr, b=J_)
                                nc.vector.tensor_add(catw[:, hr * half:hr * (half + 1), :],
                                                     lpv, t3)

                        brA, brB = branches
                        phase1(brA, do_lepe=True)
                        phase1(brB, do_lepe=False)
                        phase2(brA)
                        lepe_half(brB, 0)
                        lepe_half(brB, 1)
                        phase2(brB)

                    # ============ emission: QKV pipelined with B, then A ============
                    qkv_chunk(0)
                    qkv_chunk(1)
                    for w in range(8):
                        window_pair("B", w)
                        if w + 2 < 8:
                            qkv_chunk(w + 2)
                    for w in range(8):
                        window_pair("A", w)

                    # ============ proj^T tail ============
                    pools = [avps, dnps, auxps]
                    tags = ["av", "dn", "aux"]
                    for u, (n, oh) in enumerate([(n, oh) for n in range(8) for oh in range(2)]):
                        pool = pools[u % 3]
                        pp = pool.tile([128, 512], F32, tag=tags[u % 3], name="pp")
                        for b2 in range(4):
                            nc.tensor.matmul(pp, pw_t[b2][:, 128 * oh:128 * (oh + 1)],
                                             cat_t[b2][:, 512 * n:512 * (n + 1)],
                                             start=(b2 == 0), stop=(b2 == 3),
                                             skip_group_check=True)
                        osb = outsb.tile([128, 512], F32, tag="out", name="osb")
                        nc.vector.tensor_scalar(osb, pp, pbT_t[:, oh:oh + 1], None, ALU.add)
                        nc.sync.dma_start(out_d[128 * oh:128 * (oh + 1),
                                                512 * n:512 * (n + 1)], osb)

            if dyn_loop:
                with tc.For_i(0, dyn_loop, 1):
                    _emit(0)
            else:
                for _rep in range(repeat):
                    _emit(_rep)

    return nc


_CACHE = {}


def _get_nc(debug=False, repeat=1, dyn_loop=0, with_cbias=True, probe=frozenset()):
    key = (bool(debug), repeat, dyn_loop, with_cbias, probe)
    if key not in _CACHE:
        nc = bacc.Bacc("TRN2", target_bir_lowering=False, debug=False)
        build(nc, debug=debug, repeat=repeat, dyn_loop=dyn_loop, with_cbias=with_cbias,
              probe=probe)
        nc.compile()
        _CACHE[key] = nc
    return _CACHE[key]


def prep_inputs(x, qkv_w, proj_w, proj_b, conv_ws, conv_bs):
    x = np.asarray(x)
    B = x.shape[0]
    qwb = np.asarray(qkv_w).astype(ml_dtypes.bfloat16)
    pwb = np.asarray(proj_w).astype(ml_dtypes.bfloat16)
    w9 = np.asarray(conv_ws).reshape(4, 128, 9).astype(np.float32)
    dgn = np.zeros((36, 128, 128), np.float32)
    idx = np.arange(128)
    for br in range(4):
        for t, (dr, dj) in enumerate(TAPS):
            dgn[br * 9 + t, idx, idx] = w9[br, :, (dr + 1) * 3 + (dj + 1)]
    dgn = dgn.astype(ml_dtypes.bfloat16)
    pbTn = np.ascontiguousarray(
        np.asarray(proj_b, np.float32).reshape(2, 128).T)
    cbt = np.ascontiguousarray(np.asarray(conv_bs, np.float32).T)
    shared = {"qw": qwb, "pw": pwb, "dg": dgn, "pbT": pbTn, "cb": cbt}
    return [dict(shared,
                 xT=np.ascontiguousarray(x[b].T.astype(ml_dtypes.bfloat16)))
            for b in range(B)]


def kernel(x, qkv_w, proj_w, proj_b, conv_ws, conv_bs, _debug=False, _trace=False):
    wcb = bool(np.any(np.asarray(conv_bs)))
    nc = _get_nc(debug=_debug, with_cbias=wcb)
    in_maps = prep_inputs(x, qkv_w, proj_w, proj_b, conv_ws, conv_bs)
    res = run_bass_kernel_spmd(nc, in_maps, core_ids=list(range(len(in_maps))),
                               trace=_trace)
    out = np.stack([np.ascontiguousarray(r["out"].T) for r in res.results]
                   ).astype(np.float32)
    if _debug or _trace:
        kernel.last_results = res
    return out

